# revision 15
# baseline (speedup 1.0000x reference)
"""End2EndPoseLoss on 8 Trainium2 NeuronCores — v2.

Data-parallel over batch: each core handles B_LOC=2 samples, i.e. a
[680, 4096] fp16 pred/gt pair (11.14 MB -> ~28.6 us DMA floor at the
measured ~390 GB/s per-core aggregate).  v2 is designed to be DMA-bound:

Per row-piece (5 full [128,4096] chunks, a [128,3584]+[128,512] split
of the last full rows, and the 40 leftover rows folded to [80,2048]):

  DVE : W4 = (g > 0.2) * 4          tensor_scalar, 4x perf mode
  DVE : d  = p - g                  tensor_tensor, 2x perf mode
  ACT : D2 = Square(d)  (in-place, full output, no accumulator)
  DVE/Pool : S = rowsum((W4+1)*D2)  scalar_tensor_tensor + accum_out,
             column-split ~25/75 between DVE (1x) and the otherwise
             idle GpSimd/Pool engine (0.6 eff) -> two sums columns.

This uses exact {1,5} weights (vs sqrt(5) folding) and puts every
engine just under the DMA roofline (ACT ~26us, DVE ~28, Pool ~27).

Output path: the [128,13] f32 per-(piece,engine) row sums are
transposed on the idle PE (matmul against an on-chip identity) into
PSUM [13,128], so the final DMA is 13 512-B descriptors instead of
128 24-B ones (the baseline lost ~7 us draining those semaphores).

Small losses (count CE over [2,21], conf focal over [2,20]) arrive as
one packed [2,84] tensor, run on DVE/ACT/Pool inside the DMA ramp, and
leave as one [2,3] tensor.
"""

import sys
import types
import numpy as np

import concourse.bacc as bacc
import concourse.bass as bass  # noqa: F401
import concourse.mybir as mybir
import concourse.tile as tile
from concourse import bass_utils

# Problem constants (hardcoded per contract).
B, P, K, H, W = 16, 20, 17, 64, 64
N_CORES = 8
B_LOC = B // N_CORES            # 2
ROWS = B_LOC * P * K            # 680
COLS = H * W                    # 4096
REM = 40                        # 680 - 5*128

PEAK_THRESH = 0.2
PEAK_WEIGHT = 5.0
ALPHA_COUNT, ALPHA_HEATMAP, ALPHA_CONF = 1.0, 10.0, 1.5
EPS = 1e-6

F32 = mybir.dt.float32
F16 = mybir.dt.float16
ALU = mybir.AluOpType
ACTF = mybir.ActivationFunctionType
AX = mybir.AxisListType

# pieces: (name, rows, cols, ldN = N-cols on DVE (rest on Pool), s2_act)
# piece i accumulates Sum(d^2) in column 2i and Sum(4*step*d^2) in 2i+1.
PIECES = [
    ("t",  80, 2048,  768, False),
    ("c0", 128, 4096, 1536, False),
    ("c1", 128, 4096, 1536, False),
    ("c2", 128, 4096, 1536, True),
    ("c3", 128, 4096, 1536, False),
    ("c4a", 128, 3584, 1344, False),
    ("c4b", 128, 512,  512, False),
]
NSUM = 14


def _install_ntff_hook():
    """Provide antenv.axon_hooks if the image lacks it, so that
    run_bass_kernel_spmd(trace=True) doesn't crash and, when possible,
    actually profiles via the axon .so."""
    try:
        from antenv.axon_hooks import get_axon_ntff_profile_hook  # noqa: F401
        return
    except ImportError:
        pass
    try:
        import antenv
    except ImportError:
        return
    import contextlib
    import ctypes

    mod = types.ModuleType("antenv.axon_hooks")
    _h = [None]
    mod.set_axon_ntff_profile_hook = lambda h: _h.__setitem__(0, h)
    mod.get_axon_ntff_profile_hook = lambda: _h[0]
    sys.modules["antenv.axon_hooks"] = mod
    antenv.axon_hooks = mod

    so_path = "/opt/axon/libaxon_pjrt.so"
    try:
        lib = ctypes.CDLL(so_path)
        if not hasattr(lib, "axon_start_nrt_profile"):
            return
        lib.axon_start_nrt_profile.argtypes = [
            ctypes.POINTER(ctypes.c_int64),
            ctypes.c_size_t,
        ]
        lib.axon_start_nrt_profile.restype = ctypes.c_int64
        lib.axon_stop_nrt_profile.argtypes = [ctypes.c_char_p]
        lib.axon_stop_nrt_profile.restype = ctypes.c_int64
    except OSError:
        return

    @contextlib.contextmanager
    def _hook(output_dir, device_ids):
        import jax

        jax.devices()
        if device_ids:
            ids = (ctypes.c_int64 * len(device_ids))(*device_ids)
            rc = lib.axon_start_nrt_profile(ids, len(device_ids))
        else:
            rc = lib.axon_start_nrt_profile(None, 0)
        if rc != 0:
            raise RuntimeError(f"axon_start_nrt_profile rc={rc}")
        try:
            yield
        finally:
            n = lib.axon_stop_nrt_profile(str(output_dir).encode())
            print(f"profile: {n} file(s) written to {output_dir}", file=sys.stderr)

    mod.set_axon_ntff_profile_hook(_hook)


_install_ntff_hook()

_orig_upload = bass_utils.upload_artifacts


def _safe_upload(tmpdir):
    try:
        return _orig_upload(tmpdir)
    except Exception:
        return tmpdir


bass_utils.upload_artifacts = _safe_upload


def build_module():
    nc = bacc.Bacc("TRN2", target_bir_lowering=False, debug=False)

    srcs = {}
    for name, rr, cc, _, _ in PIECES:
        srcs["g" + name] = nc.dram_tensor("g" + name, [rr, cc], F16, kind="ExternalInput")
        srcs["p" + name] = nc.dram_tensor("p" + name, [rr, cc], F16, kind="ExternalInput")
    sm = nc.dram_tensor("sm", [B_LOC, 84], F32, kind="ExternalInput")

    out_hm = nc.dram_tensor("out_hm", [NSUM, 128], F32, kind="ExternalOutput")
    out_sm = nc.dram_tensor("out_sm", [B_LOC, 3], F32, kind="ExternalOutput")

    with tile.TileContext(nc) as tc:
        with (
            tc.tile_pool(name="io", bufs=5) as io,
            tc.tile_pool(name="wk", bufs=3) as wk,
            tc.tile_pool(name="acc", bufs=1) as accp,
            tc.tile_pool(name="small", bufs=1) as small,
            tc.tile_pool(name="ps", bufs=1, space=bass.MemorySpace.PSUM) as ps,
        ):
            sums = accp.tile([128, NSUM], F32, tag="sums")
            isrc = accp.tile([128, 128], F32, tag="isrc")
            ident = accp.tile([128, 128], F32, tag="ident")
            psum_t = ps.tile([NSUM, 128], F32, tag="pt")
            nc.gpsimd.memset(sums[:], 0.0)
            nc.gpsimd.memset(isrc[:], 1.0)
            nc.gpsimd.affine_select(
                out=ident[:], in_=isrc[:], pattern=[[-1, 128]],
                compare_op=ALU.is_equal, fill=0.0, base=0, channel_multiplier=1,
            )

            # ---- input DMA triggers, g before p so W4 can start early ----
            gts, pts = [], []
            for i, (name, rr, cc, _, _) in enumerate(PIECES):
                g = io.tile([128, COLS], F16, tag="g")
                p = io.tile([128, COLS], F16, tag="p")
                nc.sync.dma_start(g[:rr, :cc], srcs["g" + name][:, :])
                nc.sync.dma_start(p[:rr, :cc], srcs["p" + name][:, :])
                gts.append(g)
                pts.append(p)
                if i == 0:
                    smt = small.tile([B_LOC, 84], F32, tag="sm")
                    nc.sync.dma_start(smt[:], sm[:, :])

            def heavy(i):
                name, rr, cc, ldn, s2_act = PIECES[i]
                c1, c2 = 2 * i, 2 * i + 1
                g, p = gts[i], pts[i]
                w4 = wk.tile([128, COLS], F16, tag="w4")
                d = wk.tile([128, COLS], F16, tag="d")
                # W4 = (g > thresh) * 4       (TS, 4x)
                nc.vector.tensor_scalar(
                    w4[:rr, :cc], g[:rr, :cc], float(PEAK_THRESH),
                    PEAK_WEIGHT - 1.0, op0=ALU.is_gt, op1=ALU.mult,
                )
                # d = p - g                   (TT, 2x)
                nc.vector.tensor_sub(d[:rr, :cc], p[:rr, :cc], g[:rr, :cc])
                # D2 = d*d in place, Sum(d^2) rides the ACT accumulator
                nc.scalar.activation(
                    d[:rr, :cc], d[:rr, :cc], ACTF.Square,
                    accum_out=sums[:rr, c1 : c1 + 1],
                )
                # N = W4 * D2 -> p tile       (TT, split DVE / Pool)
                nc.vector.tensor_mul(p[:rr, :ldn], w4[:rr, :ldn], d[:rr, :ldn])
                if ldn < cc:
                    nc.gpsimd.tensor_mul(p[:rr, ldn:cc], w4[:rr, ldn:cc], d[:rr, ldn:cc])
                # Sum(N) = Sum(4*step*d^2)
                if s2_act:
                    nc.scalar.activation(
                        g[:rr, :cc], p[:rr, :cc], ACTF.Copy,
                        accum_out=sums[:rr, c2 : c2 + 1],
                    )
                else:
                    nc.vector.tensor_scalar(
                        g[:rr, :cc], p[:rr, :cc], 1.0, 0.0,
                        op0=ALU.mult, op1=ALU.add,
                        accum_out=sums[:rr, c2 : c2 + 1],
                    )

            # tail piece first (small, warms the pipeline during the ramp)
            heavy(0)

            # ---- small losses (inside the DMA ramp) ----
            cl_t = smt[:, 0:21]
            oh_t = smt[:, 21:42]
            lt_ = smt[:, 42:62]
            tt_ = smt[:, 62:82]

            # count cross-entropy pieces
            mx = small.tile([B_LOC, 1], F32, tag="mx")
            nc.vector.tensor_reduce(mx[:], cl_t, axis=AX.X, op=ALU.max)
            nmx = small.tile([B_LOC, 1], F32, tag="nmx")
            nc.vector.tensor_scalar_mul(nmx[:], mx[:], -1.0)
            et = small.tile([B_LOC, 21], F32, tag="et")
            se = small.tile([B_LOC, 1], F32, tag="se")
            nc.scalar.activation(
                et[:], cl_t, ACTF.Exp, bias=nmx[:], scale=1.0, accum_out=se[:]
            )
            junk21 = small.tile([B_LOC, 21], F32, tag="junk21")
            tg = small.tile([B_LOC, 1], F32, tag="tg")
            nc.vector.scalar_tensor_tensor(
                out=junk21[:], in0=cl_t, scalar=1.0, in1=oh_t,
                op0=ALU.mult, op1=ALU.mult, accum_out=tg[:],
            )
            outsm = small.tile([B_LOC, 3], F32, tag="outsm")
            nc.vector.tensor_sub(outsm[:, 0:1], mx[:], tg[:])

            # focal: p_t = 1 - |t - sigma(l)| with sigma from exp(-|l|)
            ab = small.tile([B_LOC, P], F32, tag="ab")
            nc.vector.scalar_tensor_tensor(
                out=ab[:], in0=lt_, scalar=-1.0, in1=lt_,
                op0=ALU.mult, op1=ALU.max,
            )
            z = small.tile([B_LOC, P], F32, tag="z")
            nc.scalar.activation(z[:], ab[:], ACTF.Exp, scale=-1.0)
            zz = small.tile([B_LOC, P], F32, tag="zz")
            nc.vector.tensor_scalar(zz[:], z[:], 1.0, None, op0=ALU.add)
            r = small.tile([B_LOC, P], F32, tag="r")
            nc.vector.reciprocal(r[:], zz[:])          # sigma(|l|)
            sgn = small.tile([B_LOC, P], F32, tag="sgn")
            nc.vector.tensor_scalar(sgn[:], lt_, 0.0, None, op0=ALU.is_ge)
            t1 = small.tile([B_LOC, P], F32, tag="t1")
            nc.vector.tensor_scalar(t1[:], r[:], 2.0, -1.0, op0=ALU.mult, op1=ALU.add)
            t2 = small.tile([B_LOC, P], F32, tag="t2")
            nc.vector.tensor_scalar(t2[:], r[:], -1.0, 1.0, op0=ALU.mult, op1=ALU.add)
            sl0 = small.tile([B_LOC, P], F32, tag="sl0")
            nc.vector.scalar_tensor_tensor(
                out=sl0[:], in0=sgn[:], scalar=1.0, in1=t1[:],
                op0=ALU.mult, op1=ALU.mult,
            )
            sig = small.tile([B_LOC, P], F32, tag="sig")
            nc.gpsimd.tensor_add(sig[:], sl0[:], t2[:])
            u = small.tile([B_LOC, P], F32, tag="u")
            nc.gpsimd.tensor_sub(u[:], tt_, sig[:])
            au = small.tile([B_LOC, P], F32, tag="au")
            nc.vector.scalar_tensor_tensor(
                out=au[:], in0=u[:], scalar=-1.0, in1=u[:],
                op0=ALU.mult, op1=ALU.max,
            )
            pt = small.tile([B_LOC, P], F32, tag="pt")
            nc.vector.tensor_scalar(pt[:], au[:], -1.0, 1.0, op0=ALU.mult, op1=ALU.add)
            au2 = small.tile([B_LOC, P], F32, tag="au2")
            nc.gpsimd.tensor_mul(au2[:], au[:], au[:])

            lnz = small.tile([B_LOC, 1], F32, tag="lnz")
            nc.scalar.activation(lnz[:], se[:], ACTF.Ln)
            nc.vector.tensor_copy(outsm[:, 1:2], lnz[:])
            lnpt = small.tile([B_LOC, P], F32, tag="lnpt")
            nc.scalar.activation(lnpt[:], pt[:], ACTF.Ln)
            junk20 = small.tile([B_LOC, P], F32, tag="junk20")
            fr = small.tile([B_LOC, 1], F32, tag="fr")
            # accum = sum(au^2 * ln(p_t)) = -focal_sum   (host negates)
            nc.vector.scalar_tensor_tensor(
                out=junk20[:], in0=au2[:], scalar=1.0, in1=lnpt[:],
                op0=ALU.mult, op1=ALU.mult, accum_out=fr[:],
            )
            nc.vector.tensor_copy(outsm[:, 2:3], fr[:])
            nc.sync.dma_start(out_sm[:, :], outsm[:])

            # ---- remaining heavy pieces ----
            for i in range(1, len(PIECES)):
                heavy(i)

            # ---- PE transpose of sums -> PSUM [NSUM, 128], tiny out DMA ----
            sums_t = accp.tile([NSUM, 128], F32, tag="sums_t")
            psum_t2 = ps.tile([2, 128], F32, tag="pt2")
            nc.tensor.matmul(psum_t[0:12, :], sums[:, 0:12], ident[:, :])
            nc.vector.tensor_copy(sums_t[0:12, :], psum_t[0:12, :])
            nc.sync.dma_start(out_hm[0:12, :], sums_t[0:12, :])
            sums_t2 = accp.tile([2, 128], F32, tag="sums_t2")
            nc.tensor.matmul(psum_t2[0:2, :], sums[:, 12:NSUM], ident[:, :])
            nc.vector.tensor_copy(sums_t2[0:2, :], psum_t2[0:2, :])
            nc.sync.dma_start(out_hm[12:NSUM, :], sums_t2[0:2, :])

    nc.compile()
    return nc


_MODULE = None


def _module():
    global _MODULE
    if _MODULE is None:
        _MODULE = build_module()
    return _MODULE


def make_in_maps(count_logits, pred_heatmaps, pred_conf_logits, gt_heatmaps,
                 count, mask):
    count_logits = np.asarray(count_logits, np.float32)
    pred_heatmaps = np.asarray(pred_heatmaps, np.float32)
    pred_conf_logits = np.asarray(pred_conf_logits, np.float32)
    gt_heatmaps = np.asarray(gt_heatmaps, np.float32)
    count = np.asarray(count, np.int32)
    mask = np.asarray(mask, np.int32)

    in_maps = []
    for i in range(N_CORES):
        b0, b1 = i * B_LOC, (i + 1) * B_LOC
        phl = pred_heatmaps[b0:b1].reshape(ROWS, COLS).astype(np.float16)
        ghl = gt_heatmaps[b0:b1].reshape(ROWS, COLS).astype(np.float16)

        im = {}
        for name, rr, cc, _, _ in PIECES:
            if name == "t":
                # rows 640:680 folded: partition h*40+r <-> row 640+r, half h
                pt_ = phl[640:].reshape(REM, 2, COLS // 2).transpose(1, 0, 2)
                gt_ = ghl[640:].reshape(REM, 2, COLS // 2).transpose(1, 0, 2)
                im["pt"] = np.ascontiguousarray(pt_.reshape(2 * REM, COLS // 2))
                im["gt"] = np.ascontiguousarray(gt_.reshape(2 * REM, COLS // 2))
            else:
                r0 = 512 if name in ("c4a", "c4b") else int(name[1]) * 128
                c0 = 0
                if name == "c4b":
                    c0 = 3584
                im["p" + name] = np.ascontiguousarray(phl[r0 : r0 + rr, c0 : c0 + cc])
                im["g" + name] = np.ascontiguousarray(ghl[r0 : r0 + rr, c0 : c0 + cc])

        smv = np.zeros((B_LOC, 84), np.float32)
        smv[np.arange(B_LOC), 21 + count[b0:b1]] = 1.0       # one-hot
        smv[:, 0:21] = count_logits[b0:b1]
        smv[:, 42:62] = pred_conf_logits[b0:b1]
        smv[:, 62:82] = mask[b0:b1].astype(np.float32)
        im["sm"] = smv
        in_maps.append(im)
    return in_maps


def _rowsums(hm):
    """[NSUM, 128] transposed sums -> [680] per-row weighted sums.

    Piece i's Sum(d^2) is row 2i, Sum(4*step*d^2) is row 2i+1; their sum
    is the per-(piece, partition) weighted heatmap contribution."""
    s = hm[0::2] + hm[1::2]          # [7, 128]
    rows = np.zeros(ROWS)
    rows[640:680] = s[0, :80].reshape(2, REM).sum(axis=0)
    for k in range(4):
        rows[128 * k : 128 * (k + 1)] = s[1 + k]
    rows[512:640] = s[5] + s[6]
    return rows


def combine(results, mask):
    mask = np.asarray(mask)
    hm_sum = 0.0
    ce_sum = 0.0
    fo_sum = 0.0
    for i, res in enumerate(results):
        b0, b1 = i * B_LOC, (i + 1) * B_LOC
        rowsum = _rowsums(np.asarray(res["out_hm"], np.float64))
        mrow = np.repeat(mask[b0:b1].astype(np.float64).reshape(-1), K)
        hm_sum += float(rowsum @ mrow)
        sm = np.asarray(res["out_sm"], np.float64)       # [2,3]: pre, lnz, fr
        ce_sum += float(sm[:, 0:2].sum())
        fo_sum += -float(sm[:, 2].sum())
    msum = float(mask.sum())
    hm = hm_sum / (msum * K * H * W + EPS)
    loss_heatmap = hm if msum > 0 else 0.0
    loss_count = ce_sum / B
    loss_conf = fo_sum / (B * P)
    total = (ALPHA_COUNT * loss_count + ALPHA_HEATMAP * loss_heatmap
             + ALPHA_CONF * loss_conf)
    return np.float32(total)


def run(inputs, trace=False, **kwargs):
    """Run on hardware; returns (output_scalar, BassKernelResults)."""
    nc = _module()
    in_maps = make_in_maps(**inputs)
    res = bass_utils.run_bass_kernel_spmd(
        nc, in_maps, core_ids=list(range(N_CORES)), trace=trace, **kwargs
    )
    out = combine(res.results, inputs["mask"])
    return out, res


def kernel(count_logits, pred_heatmaps, pred_conf_logits, gt_heatmaps,
           count, mask):
    out, _ = run(dict(
        count_logits=count_logits, pred_heatmaps=pred_heatmaps,
        pred_conf_logits=pred_conf_logits, gt_heatmaps=gt_heatmaps,
        count=count, mask=mask,
    ))
    return out


# revision 18
# speedup vs baseline: 1.2386x; 1.2386x over previous
"""End2EndPoseLoss on 8 Trainium2 NeuronCores — v2.

Data-parallel over batch: each core handles B_LOC=2 samples, i.e. a
[680, 4096] fp16 pred/gt pair (11.14 MB -> ~28.6 us DMA floor at the
measured ~390 GB/s per-core aggregate).  v2 is designed to be DMA-bound:

Per row-piece (5 full [128,4096] chunks, a [128,3584]+[128,512] split
of the last full rows, and the 40 leftover rows folded to [80,2048]):

  DVE : W4 = (g > 0.2) * 4          tensor_scalar, 4x perf mode
  DVE : d  = p - g                  tensor_tensor, 2x perf mode
  ACT : D2 = Square(d)  (in-place, full output, no accumulator)
  DVE/Pool : S = rowsum((W4+1)*D2)  scalar_tensor_tensor + accum_out,
             column-split ~25/75 between DVE (1x) and the otherwise
             idle GpSimd/Pool engine (0.6 eff) -> two sums columns.

This uses exact {1,5} weights (vs sqrt(5) folding) and puts every
engine just under the DMA roofline (ACT ~26us, DVE ~28, Pool ~27).

Output path: the [128,13] f32 per-(piece,engine) row sums are
transposed on the idle PE (matmul against an on-chip identity) into
PSUM [13,128], so the final DMA is 13 512-B descriptors instead of
128 24-B ones (the baseline lost ~7 us draining those semaphores).

Small losses (count CE over [2,21], conf focal over [2,20]) arrive as
one packed [2,84] tensor, run on DVE/ACT/Pool inside the DMA ramp, and
leave as one [2,3] tensor.
"""

import sys
import types
import numpy as np

import concourse.bacc as bacc
import concourse.bass as bass  # noqa: F401
import concourse.mybir as mybir
import concourse.tile as tile
from concourse import bass_utils

# Problem constants (hardcoded per contract).
B, P, K, H, W = 16, 20, 17, 64, 64
N_CORES = 8
B_LOC = B // N_CORES            # 2
ROWS = B_LOC * P * K            # 680
COLS = H * W                    # 4096
REM = 40                        # 680 - 5*128

PEAK_THRESH = 0.2
PEAK_WEIGHT = 5.0
ALPHA_COUNT, ALPHA_HEATMAP, ALPHA_CONF = 1.0, 10.0, 1.5
EPS = 1e-6

F32 = mybir.dt.float32
F16 = mybir.dt.float16
ALU = mybir.AluOpType
ACTF = mybir.ActivationFunctionType
AX = mybir.AxisListType

# pieces: (name, rows, cols, s2_dve = Sum2 via DVE stt instead of ACT square)
# piece i accumulates Sum(d^2) in column 2i and 4*Sum(step*d^2) in 2i+1.
PIECES = [
    ("t",  80, 2048,  True),
    ("c0", 128, 4096, True),
    ("c1", 128, 4096, False),
    ("c2", 128, 4096, False),
    ("c3", 128, 4096, False),
    ("c4a", 128, 3584, False),
    ("c4b", 128, 512,  True),
]
NSUM = 14


def _install_ntff_hook():
    """Provide antenv.axon_hooks if the image lacks it, so that
    run_bass_kernel_spmd(trace=True) doesn't crash and, when possible,
    actually profiles via the axon .so."""
    try:
        from antenv.axon_hooks import get_axon_ntff_profile_hook  # noqa: F401
        return
    except ImportError:
        pass
    try:
        import antenv
    except ImportError:
        return
    import contextlib
    import ctypes

    mod = types.ModuleType("antenv.axon_hooks")
    _h = [None]
    mod.set_axon_ntff_profile_hook = lambda h: _h.__setitem__(0, h)
    mod.get_axon_ntff_profile_hook = lambda: _h[0]
    sys.modules["antenv.axon_hooks"] = mod
    antenv.axon_hooks = mod

    so_path = "/opt/axon/libaxon_pjrt.so"
    try:
        lib = ctypes.CDLL(so_path)
        if not hasattr(lib, "axon_start_nrt_profile"):
            return
        lib.axon_start_nrt_profile.argtypes = [
            ctypes.POINTER(ctypes.c_int64),
            ctypes.c_size_t,
        ]
        lib.axon_start_nrt_profile.restype = ctypes.c_int64
        lib.axon_stop_nrt_profile.argtypes = [ctypes.c_char_p]
        lib.axon_stop_nrt_profile.restype = ctypes.c_int64
    except OSError:
        return

    @contextlib.contextmanager
    def _hook(output_dir, device_ids):
        import jax

        jax.devices()
        if device_ids:
            ids = (ctypes.c_int64 * len(device_ids))(*device_ids)
            rc = lib.axon_start_nrt_profile(ids, len(device_ids))
        else:
            rc = lib.axon_start_nrt_profile(None, 0)
        if rc != 0:
            raise RuntimeError(f"axon_start_nrt_profile rc={rc}")
        try:
            yield
        finally:
            n = lib.axon_stop_nrt_profile(str(output_dir).encode())
            print(f"profile: {n} file(s) written to {output_dir}", file=sys.stderr)

    mod.set_axon_ntff_profile_hook(_hook)


_install_ntff_hook()

_orig_upload = bass_utils.upload_artifacts


def _safe_upload(tmpdir):
    try:
        return _orig_upload(tmpdir)
    except Exception:
        return tmpdir


bass_utils.upload_artifacts = _safe_upload


def build_module():
    nc = bacc.Bacc("TRN2", target_bir_lowering=False, debug=False)

    srcs = {}
    for name, rr, cc, _ in PIECES:
        srcs["g" + name] = nc.dram_tensor("g" + name, [rr, cc], F16, kind="ExternalInput")
        srcs["p" + name] = nc.dram_tensor("p" + name, [rr, cc], F16, kind="ExternalInput")
    sm = nc.dram_tensor("sm", [B_LOC, 84], F32, kind="ExternalInput")

    out_hm = nc.dram_tensor("out_hm", [NSUM, 128], F32, kind="ExternalOutput")
    out_sm = nc.dram_tensor("out_sm", [B_LOC, 3], F32, kind="ExternalOutput")

    with tile.TileContext(nc) as tc:
        with (
            tc.tile_pool(name="io", bufs=5) as io,
            tc.tile_pool(name="wk", bufs=3) as wk,
            tc.tile_pool(name="acc", bufs=1) as accp,
            tc.tile_pool(name="small", bufs=1) as small,
            tc.tile_pool(name="ps", bufs=1, space=bass.MemorySpace.PSUM) as ps,
        ):
            sums = accp.tile([128, NSUM], F32, tag="sums")
            isrc = accp.tile([128, 128], F32, tag="isrc")
            ident = accp.tile([128, 128], F32, tag="ident")
            psum_t = ps.tile([NSUM, 128], F32, tag="pt")
            nc.gpsimd.memset(sums[:], 0.0)
            nc.gpsimd.memset(isrc[:], 1.0)
            nc.gpsimd.affine_select(
                out=ident[:], in_=isrc[:], pattern=[[-1, 128]],
                compare_op=ALU.is_equal, fill=0.0, base=0, channel_multiplier=1,
            )

            # ---- input DMA triggers, g before p so W4 can start early ----
            gts, pts = [], []
            for i, (name, rr, cc, _) in enumerate(PIECES):
                g = io.tile([128, COLS], F16, tag="g")
                p = io.tile([128, COLS], F16, tag="p")
                nc.sync.dma_start(g[:rr, :cc], srcs["g" + name][:, :])
                nc.sync.dma_start(p[:rr, :cc], srcs["p" + name][:, :])
                gts.append(g)
                pts.append(p)
                if i == 0:
                    smt = small.tile([B_LOC, 84], F32, tag="sm")
                    nc.sync.dma_start(smt[:], sm[:, :])

            def heavy(i):
                name, rr, cc, s2_dve = PIECES[i]
                c1, c2 = 2 * i, 2 * i + 1
                g, p = gts[i], pts[i]
                st = wk.tile([128, COLS], F16, tag="st")
                d = wk.tile([128, COLS], F16, tag="d")
                n = wk.tile([128, COLS], F16, tag="n")
                # step = (g > thresh)          (TS, 4x)
                nc.vector.tensor_scalar(
                    st[:rr, :cc], g[:rr, :cc], float(PEAK_THRESH), None,
                    op0=ALU.is_gt,
                )
                # d = p - g                    (TT, 2x)
                nc.vector.tensor_sub(d[:rr, :cc], p[:rr, :cc], g[:rr, :cc])
                # n = step * d                 (TT, 2x)
                nc.vector.tensor_mul(n[:rr, :cc], st[:rr, :cc], d[:rr, :cc])
                # Sum(d^2) on ACT (in-place square, accumulator -> col 2i)
                nc.scalar.activation(
                    d[:rr, :cc], d[:rr, :cc], ACTF.Square,
                    accum_out=sums[:rr, c1 : c1 + 1],
                )
                # 4*Sum((step*d)^2) -> col 2i+1: ACT square of 2n, or DVE stt
                if s2_dve:
                    nc.vector.scalar_tensor_tensor(
                        out=g[:rr, :cc], in0=n[:rr, :cc], scalar=4.0,
                        in1=n[:rr, :cc], op0=ALU.mult, op1=ALU.mult,
                        accum_out=sums[:rr, c2 : c2 + 1],
                    )
                else:
                    nc.scalar.activation(
                        n[:rr, :cc], n[:rr, :cc], ACTF.Square, scale=2.0,
                        accum_out=sums[:rr, c2 : c2 + 1],
                    )

            # tail piece first (small, warms the pipeline during the ramp)
            heavy(0)

            # ---- small losses (inside the DMA ramp) ----
            cl_t = smt[:, 0:21]
            oh_t = smt[:, 21:42]
            lt_ = smt[:, 42:62]
            tt_ = smt[:, 62:82]

            # count cross-entropy pieces
            mx = small.tile([B_LOC, 1], F32, tag="mx")
            nc.vector.tensor_reduce(mx[:], cl_t, axis=AX.X, op=ALU.max)
            nmx = small.tile([B_LOC, 1], F32, tag="nmx")
            nc.vector.tensor_scalar_mul(nmx[:], mx[:], -1.0)
            et = small.tile([B_LOC, 21], F32, tag="et")
            se = small.tile([B_LOC, 1], F32, tag="se")
            nc.scalar.activation(
                et[:], cl_t, ACTF.Exp, bias=nmx[:], scale=1.0, accum_out=se[:]
            )
            junk21 = small.tile([B_LOC, 21], F32, tag="junk21")
            tg = small.tile([B_LOC, 1], F32, tag="tg")
            nc.vector.scalar_tensor_tensor(
                out=junk21[:], in0=cl_t, scalar=1.0, in1=oh_t,
                op0=ALU.mult, op1=ALU.mult, accum_out=tg[:],
            )
            outsm = small.tile([B_LOC, 3], F32, tag="outsm")
            nc.vector.tensor_sub(outsm[:, 0:1], mx[:], tg[:])

            # focal: p_t = 1 - |t - sigma(l)| with sigma from exp(-|l|)
            ab = small.tile([B_LOC, P], F32, tag="ab")
            nc.vector.scalar_tensor_tensor(
                out=ab[:], in0=lt_, scalar=-1.0, in1=lt_,
                op0=ALU.mult, op1=ALU.max,
            )
            z = small.tile([B_LOC, P], F32, tag="z")
            nc.scalar.activation(z[:], ab[:], ACTF.Exp, scale=-1.0)
            zz = small.tile([B_LOC, P], F32, tag="zz")
            nc.vector.tensor_scalar(zz[:], z[:], 1.0, None, op0=ALU.add)
            r = small.tile([B_LOC, P], F32, tag="r")
            nc.vector.reciprocal(r[:], zz[:])          # sigma(|l|)
            sgn = small.tile([B_LOC, P], F32, tag="sgn")
            nc.vector.tensor_scalar(sgn[:], lt_, 0.0, None, op0=ALU.is_ge)
            t1 = small.tile([B_LOC, P], F32, tag="t1")
            nc.vector.tensor_scalar(t1[:], r[:], 2.0, -1.0, op0=ALU.mult, op1=ALU.add)
            t2 = small.tile([B_LOC, P], F32, tag="t2")
            nc.vector.tensor_scalar(t2[:], r[:], -1.0, 1.0, op0=ALU.mult, op1=ALU.add)
            sl0 = small.tile([B_LOC, P], F32, tag="sl0")
            nc.vector.scalar_tensor_tensor(
                out=sl0[:], in0=sgn[:], scalar=1.0, in1=t1[:],
                op0=ALU.mult, op1=ALU.mult,
            )
            sig = small.tile([B_LOC, P], F32, tag="sig")
            nc.gpsimd.tensor_add(sig[:], sl0[:], t2[:])
            u = small.tile([B_LOC, P], F32, tag="u")
            nc.gpsimd.tensor_sub(u[:], tt_, sig[:])
            au = small.tile([B_LOC, P], F32, tag="au")
            nc.vector.scalar_tensor_tensor(
                out=au[:], in0=u[:], scalar=-1.0, in1=u[:],
                op0=ALU.mult, op1=ALU.max,
            )
            pt = small.tile([B_LOC, P], F32, tag="pt")
            nc.vector.tensor_scalar(pt[:], au[:], -1.0, 1.0, op0=ALU.mult, op1=ALU.add)
            au2 = small.tile([B_LOC, P], F32, tag="au2")
            nc.gpsimd.tensor_mul(au2[:], au[:], au[:])

            lnz = small.tile([B_LOC, 1], F32, tag="lnz")
            nc.scalar.activation(lnz[:], se[:], ACTF.Ln)
            nc.vector.tensor_copy(outsm[:, 1:2], lnz[:])
            lnpt = small.tile([B_LOC, P], F32, tag="lnpt")
            nc.scalar.activation(lnpt[:], pt[:], ACTF.Ln)
            junk20 = small.tile([B_LOC, P], F32, tag="junk20")
            fr = small.tile([B_LOC, 1], F32, tag="fr")
            # accum = sum(au^2 * ln(p_t)) = -focal_sum   (host negates)
            nc.vector.scalar_tensor_tensor(
                out=junk20[:], in0=au2[:], scalar=1.0, in1=lnpt[:],
                op0=ALU.mult, op1=ALU.mult, accum_out=fr[:],
            )
            nc.vector.tensor_copy(outsm[:, 2:3], fr[:])
            nc.sync.dma_start(out_sm[:, :], outsm[:])

            # ---- remaining heavy pieces ----
            for i in range(1, len(PIECES)):
                heavy(i)

            # ---- PE transpose of sums -> PSUM [NSUM, 128], tiny out DMA ----
            sums_t = accp.tile([NSUM, 128], F32, tag="sums_t")
            psum_t2 = ps.tile([2, 128], F32, tag="pt2")
            nc.tensor.matmul(psum_t[0:12, :], sums[:, 0:12], ident[:, :])
            nc.vector.tensor_copy(sums_t[0:12, :], psum_t[0:12, :])
            nc.sync.dma_start(out_hm[0:12, :], sums_t[0:12, :])
            sums_t2 = accp.tile([2, 128], F32, tag="sums_t2")
            nc.tensor.matmul(psum_t2[0:2, :], sums[:, 12:NSUM], ident[:, :])
            nc.vector.tensor_copy(sums_t2[0:2, :], psum_t2[0:2, :])
            nc.sync.dma_start(out_hm[12:NSUM, :], sums_t2[0:2, :])

    nc.compile()
    return nc


_MODULE = None


def _module():
    global _MODULE
    if _MODULE is None:
        _MODULE = build_module()
    return _MODULE


def make_in_maps(count_logits, pred_heatmaps, pred_conf_logits, gt_heatmaps,
                 count, mask):
    count_logits = np.asarray(count_logits, np.float32)
    pred_heatmaps = np.asarray(pred_heatmaps, np.float32)
    pred_conf_logits = np.asarray(pred_conf_logits, np.float32)
    gt_heatmaps = np.asarray(gt_heatmaps, np.float32)
    count = np.asarray(count, np.int32)
    mask = np.asarray(mask, np.int32)

    in_maps = []
    for i in range(N_CORES):
        b0, b1 = i * B_LOC, (i + 1) * B_LOC
        phl = pred_heatmaps[b0:b1].reshape(ROWS, COLS).astype(np.float16)
        ghl = gt_heatmaps[b0:b1].reshape(ROWS, COLS).astype(np.float16)

        im = {}
        for name, rr, cc, _ in PIECES:
            if name == "t":
                # rows 640:680 folded: partition h*40+r <-> row 640+r, half h
                pt_ = phl[640:].reshape(REM, 2, COLS // 2).transpose(1, 0, 2)
                gt_ = ghl[640:].reshape(REM, 2, COLS // 2).transpose(1, 0, 2)
                im["pt"] = np.ascontiguousarray(pt_.reshape(2 * REM, COLS // 2))
                im["gt"] = np.ascontiguousarray(gt_.reshape(2 * REM, COLS // 2))
            else:
                r0 = 512 if name in ("c4a", "c4b") else int(name[1]) * 128
                c0 = 0
                if name == "c4b":
                    c0 = 3584
                im["p" + name] = np.ascontiguousarray(phl[r0 : r0 + rr, c0 : c0 + cc])
                im["g" + name] = np.ascontiguousarray(ghl[r0 : r0 + rr, c0 : c0 + cc])

        smv = np.zeros((B_LOC, 84), np.float32)
        smv[np.arange(B_LOC), 21 + count[b0:b1]] = 1.0       # one-hot
        smv[:, 0:21] = count_logits[b0:b1]
        smv[:, 42:62] = pred_conf_logits[b0:b1]
        smv[:, 62:82] = mask[b0:b1].astype(np.float32)
        im["sm"] = smv
        in_maps.append(im)
    return in_maps


def _rowsums(hm):
    """[NSUM, 128] transposed sums -> [680] per-row weighted sums.

    Piece i's Sum(d^2) is row 2i, Sum(4*step*d^2) is row 2i+1; their sum
    is the per-(piece, partition) weighted heatmap contribution."""
    s = hm[0::2] + hm[1::2]          # [7, 128]
    rows = np.zeros(ROWS)
    rows[640:680] = s[0, :80].reshape(2, REM).sum(axis=0)
    for k in range(4):
        rows[128 * k : 128 * (k + 1)] = s[1 + k]
    rows[512:640] = s[5] + s[6]
    return rows


def combine(results, mask):
    mask = np.asarray(mask)
    hm_sum = 0.0
    ce_sum = 0.0
    fo_sum = 0.0
    for i, res in enumerate(results):
        b0, b1 = i * B_LOC, (i + 1) * B_LOC
        rowsum = _rowsums(np.asarray(res["out_hm"], np.float64))
        mrow = np.repeat(mask[b0:b1].astype(np.float64).reshape(-1), K)
        hm_sum += float(rowsum @ mrow)
        sm = np.asarray(res["out_sm"], np.float64)       # [2,3]: pre, lnz, fr
        ce_sum += float(sm[:, 0:2].sum())
        fo_sum += -float(sm[:, 2].sum())
    msum = float(mask.sum())
    hm = hm_sum / (msum * K * H * W + EPS)
    loss_heatmap = hm if msum > 0 else 0.0
    loss_count = ce_sum / B
    loss_conf = fo_sum / (B * P)
    total = (ALPHA_COUNT * loss_count + ALPHA_HEATMAP * loss_heatmap
             + ALPHA_CONF * loss_conf)
    return np.float32(total)


def run(inputs, trace=False, **kwargs):
    """Run on hardware; returns (output_scalar, BassKernelResults)."""
    nc = _module()
    in_maps = make_in_maps(**inputs)
    res = bass_utils.run_bass_kernel_spmd(
        nc, in_maps, core_ids=list(range(N_CORES)), trace=trace, **kwargs
    )
    out = combine(res.results, inputs["mask"])
    return out, res


def kernel(count_logits, pred_heatmaps, pred_conf_logits, gt_heatmaps,
           count, mask):
    out, _ = run(dict(
        count_logits=count_logits, pred_heatmaps=pred_heatmaps,
        pred_conf_logits=pred_conf_logits, gt_heatmaps=gt_heatmaps,
        count=count, mask=mask,
    ))
    return out


# revision 22
# speedup vs baseline: 1.3277x; 1.0719x over previous
"""End2EndPoseLoss on 8 Trainium2 NeuronCores — v2.

Data-parallel over batch: each core handles B_LOC=2 samples, i.e. a
[680, 4096] fp16 pred/gt pair (11.14 MB -> ~28.6 us DMA floor at the
measured ~390 GB/s per-core aggregate).  v2 is designed to be DMA-bound:

Per row-piece (5 full [128,4096] chunks, a [128,3584]+[128,512] split
of the last full rows, and the 40 leftover rows folded to [80,2048]):

  DVE : W4 = (g > 0.2) * 4          tensor_scalar, 4x perf mode
  DVE : d  = p - g                  tensor_tensor, 2x perf mode
  ACT : D2 = Square(d)  (in-place, full output, no accumulator)
  DVE/Pool : S = rowsum((W4+1)*D2)  scalar_tensor_tensor + accum_out,
             column-split ~25/75 between DVE (1x) and the otherwise
             idle GpSimd/Pool engine (0.6 eff) -> two sums columns.

This uses exact {1,5} weights (vs sqrt(5) folding) and puts every
engine just under the DMA roofline (ACT ~26us, DVE ~28, Pool ~27).

Output path: the [128,13] f32 per-(piece,engine) row sums are
transposed on the idle PE (matmul against an on-chip identity) into
PSUM [13,128], so the final DMA is 13 512-B descriptors instead of
128 24-B ones (the baseline lost ~7 us draining those semaphores).

Small losses (count CE over [2,21], conf focal over [2,20]) arrive as
one packed [2,84] tensor, run on DVE/ACT/Pool inside the DMA ramp, and
leave as one [2,3] tensor.
"""

import sys
import types
import numpy as np

import concourse.bacc as bacc
import concourse.bass as bass  # noqa: F401
import concourse.mybir as mybir
import concourse.tile as tile
from concourse import bass_utils

# Problem constants (hardcoded per contract).
B, P, K, H, W = 16, 20, 17, 64, 64
N_CORES = 8
B_LOC = B // N_CORES            # 2
ROWS = B_LOC * P * K            # 680
COLS = H * W                    # 4096
REM = 40                        # 680 - 5*128

PEAK_THRESH = 0.2
PEAK_WEIGHT = 5.0
ALPHA_COUNT, ALPHA_HEATMAP, ALPHA_CONF = 1.0, 10.0, 1.5
EPS = 1e-6

F32 = mybir.dt.float32
F16 = mybir.dt.float16
ALU = mybir.AluOpType
ACTF = mybir.ActivationFunctionType
AX = mybir.AxisListType

# pieces: (name, rows, cols, s2_dve = Sum2 via DVE stt instead of ACT square)
# piece i accumulates Sum(d^2) in column 2i and 4*Sum(step*d^2) in 2i+1.
PIECES = [
    ("t",  80, 2048,  False),
    ("c0", 128, 4096, False),
    ("c1", 128, 4096, False),
    ("c2", 128, 4096, True),
    ("c3", 128, 4096, True),
    ("c4a", 128, 3584, True),
    ("c4b", 128, 512,  False),
]
NSUM = 14


def _install_ntff_hook():
    """Provide antenv.axon_hooks if the image lacks it, so that
    run_bass_kernel_spmd(trace=True) doesn't crash and, when possible,
    actually profiles via the axon .so."""
    try:
        from antenv.axon_hooks import get_axon_ntff_profile_hook  # noqa: F401
        return
    except ImportError:
        pass
    try:
        import antenv
    except ImportError:
        return
    import contextlib
    import ctypes

    mod = types.ModuleType("antenv.axon_hooks")
    _h = [None]
    mod.set_axon_ntff_profile_hook = lambda h: _h.__setitem__(0, h)
    mod.get_axon_ntff_profile_hook = lambda: _h[0]
    sys.modules["antenv.axon_hooks"] = mod
    antenv.axon_hooks = mod

    so_path = "/opt/axon/libaxon_pjrt.so"
    try:
        lib = ctypes.CDLL(so_path)
        if not hasattr(lib, "axon_start_nrt_profile"):
            return
        lib.axon_start_nrt_profile.argtypes = [
            ctypes.POINTER(ctypes.c_int64),
            ctypes.c_size_t,
        ]
        lib.axon_start_nrt_profile.restype = ctypes.c_int64
        lib.axon_stop_nrt_profile.argtypes = [ctypes.c_char_p]
        lib.axon_stop_nrt_profile.restype = ctypes.c_int64
    except OSError:
        return

    @contextlib.contextmanager
    def _hook(output_dir, device_ids):
        import jax

        jax.devices()
        if device_ids:
            ids = (ctypes.c_int64 * len(device_ids))(*device_ids)
            rc = lib.axon_start_nrt_profile(ids, len(device_ids))
        else:
            rc = lib.axon_start_nrt_profile(None, 0)
        if rc != 0:
            raise RuntimeError(f"axon_start_nrt_profile rc={rc}")
        try:
            yield
        finally:
            n = lib.axon_stop_nrt_profile(str(output_dir).encode())
            print(f"profile: {n} file(s) written to {output_dir}", file=sys.stderr)

    mod.set_axon_ntff_profile_hook(_hook)


_install_ntff_hook()

_orig_upload = bass_utils.upload_artifacts


def _safe_upload(tmpdir):
    try:
        return _orig_upload(tmpdir)
    except Exception:
        return tmpdir


bass_utils.upload_artifacts = _safe_upload


def build_module():
    nc = bacc.Bacc("TRN2", target_bir_lowering=False, debug=False)

    srcs = {}
    for name, rr, cc, _ in PIECES:
        srcs["g" + name] = nc.dram_tensor("g" + name, [rr, cc], F16, kind="ExternalInput")
        srcs["p" + name] = nc.dram_tensor("p" + name, [rr, cc], F16, kind="ExternalInput")
    sm = nc.dram_tensor("sm", [B_LOC, 84], F32, kind="ExternalInput")

    out_hm = nc.dram_tensor("out_hm", [NSUM, 128], F32, kind="ExternalOutput")
    out_sm = nc.dram_tensor("out_sm", [B_LOC, 3], F32, kind="ExternalOutput")

    with tile.TileContext(nc) as tc:
        with (
            tc.tile_pool(name="io", bufs=5) as io,
            tc.tile_pool(name="wk", bufs=4) as wk,
            tc.tile_pool(name="acc", bufs=1) as accp,
            tc.tile_pool(name="small", bufs=1) as small,
            tc.tile_pool(name="ps", bufs=1, space=bass.MemorySpace.PSUM) as ps,
        ):
            sums = accp.tile([128, NSUM], F32, tag="sums")
            isrc = accp.tile([128, 128], F32, tag="isrc")
            ident = accp.tile([128, 128], F32, tag="ident")
            psum_t = ps.tile([NSUM, 128], F32, tag="pt")
            nc.gpsimd.memset(sums[:], 0.0)
            nc.gpsimd.memset(isrc[:], 1.0)
            nc.gpsimd.affine_select(
                out=ident[:], in_=isrc[:], pattern=[[-1, 128]],
                compare_op=ALU.is_equal, fill=0.0, base=0, channel_multiplier=1,
            )

            # ---- input DMA triggers, g before p so W4 can start early ----
            gts, pts = [], []
            for i, (name, rr, cc, _) in enumerate(PIECES):
                g = io.tile([128, COLS], F16, tag="g")
                p = io.tile([128, COLS], F16, tag="p")
                nc.sync.dma_start(g[:rr, :cc], srcs["g" + name][:, :])
                nc.sync.dma_start(p[:rr, :cc], srcs["p" + name][:, :])
                gts.append(g)
                pts.append(p)
                if i == 0:
                    smt = small.tile([B_LOC, 84], F32, tag="sm")
                    nc.sync.dma_start(smt[:], sm[:, :])

            def heavy(i):
                name, rr, cc, s2_dve = PIECES[i]
                c1, c2 = 2 * i, 2 * i + 1
                g, p = gts[i], pts[i]
                st = wk.tile([128, COLS], F16, tag="st")
                d = wk.tile([128, COLS], F16, tag="d")
                n = wk.tile([128, COLS], F16, tag="n")
                # step = (g > thresh)          (TS, 4x)
                nc.vector.tensor_scalar(
                    st[:rr, :cc], g[:rr, :cc], float(PEAK_THRESH), None,
                    op0=ALU.is_gt,
                )
                # d = p - g                    (TT, 2x)
                nc.vector.tensor_sub(d[:rr, :cc], p[:rr, :cc], g[:rr, :cc])
                # n = step * d                 (TT, 2x)
                nc.vector.tensor_mul(n[:rr, :cc], st[:rr, :cc], d[:rr, :cc])
                # Sum(d^2) on ACT (in-place square, accumulator -> col 2i)
                nc.scalar.activation(
                    d[:rr, :cc], d[:rr, :cc], ACTF.Square,
                    accum_out=sums[:rr, c1 : c1 + 1],
                )
                # 4*Sum((step*d)^2) -> col 2i+1: ACT square of 2n, or DVE stt
                if s2_dve:
                    nc.vector.scalar_tensor_tensor(
                        out=g[:rr, :cc], in0=n[:rr, :cc], scalar=4.0,
                        in1=n[:rr, :cc], op0=ALU.mult, op1=ALU.mult,
                        accum_out=sums[:rr, c2 : c2 + 1],
                    )
                else:
                    nc.scalar.activation(
                        n[:rr, :cc], n[:rr, :cc], ACTF.Square, scale=2.0,
                        accum_out=sums[:rr, c2 : c2 + 1],
                    )

            # first three pieces keep DVE/ACT fed through the ramp; the
            # small losses interleave after them
            heavy(0)
            heavy(1)
            heavy(2)

            # ---- small losses (inside the DMA stream) ----
            cl_t = smt[:, 0:21]
            oh_t = smt[:, 21:42]
            lt_ = smt[:, 42:62]
            tt_ = smt[:, 62:82]

            # count cross-entropy pieces
            mx = small.tile([B_LOC, 1], F32, tag="mx")
            nc.vector.tensor_reduce(mx[:], cl_t, axis=AX.X, op=ALU.max)
            nmx = small.tile([B_LOC, 1], F32, tag="nmx")
            nc.vector.tensor_scalar_mul(nmx[:], mx[:], -1.0)
            et = small.tile([B_LOC, 21], F32, tag="et")
            se = small.tile([B_LOC, 1], F32, tag="se")
            nc.scalar.activation(
                et[:], cl_t, ACTF.Exp, bias=nmx[:], scale=1.0, accum_out=se[:]
            )
            junk21 = small.tile([B_LOC, 21], F32, tag="junk21")
            tg = small.tile([B_LOC, 1], F32, tag="tg")
            nc.vector.scalar_tensor_tensor(
                out=junk21[:], in0=cl_t, scalar=1.0, in1=oh_t,
                op0=ALU.mult, op1=ALU.mult, accum_out=tg[:],
            )
            outsm = small.tile([B_LOC, 3], F32, tag="outsm")
            nc.vector.tensor_sub(outsm[:, 0:1], mx[:], tg[:])

            # focal: p_t = 1 - |t - sigma(l)| with sigma from exp(-|l|)
            ab = small.tile([B_LOC, P], F32, tag="ab")
            nc.vector.scalar_tensor_tensor(
                out=ab[:], in0=lt_, scalar=-1.0, in1=lt_,
                op0=ALU.mult, op1=ALU.max,
            )
            z = small.tile([B_LOC, P], F32, tag="z")
            nc.scalar.activation(z[:], ab[:], ACTF.Exp, scale=-1.0)
            zz = small.tile([B_LOC, P], F32, tag="zz")
            nc.vector.tensor_scalar(zz[:], z[:], 1.0, None, op0=ALU.add)
            r = small.tile([B_LOC, P], F32, tag="r")
            nc.vector.reciprocal(r[:], zz[:])          # sigma(|l|)
            sgn = small.tile([B_LOC, P], F32, tag="sgn")
            nc.vector.tensor_scalar(sgn[:], lt_, 0.0, None, op0=ALU.is_ge)
            t1 = small.tile([B_LOC, P], F32, tag="t1")
            nc.vector.tensor_scalar(t1[:], r[:], 2.0, -1.0, op0=ALU.mult, op1=ALU.add)
            t2 = small.tile([B_LOC, P], F32, tag="t2")
            nc.vector.tensor_scalar(t2[:], r[:], -1.0, 1.0, op0=ALU.mult, op1=ALU.add)
            sl0 = small.tile([B_LOC, P], F32, tag="sl0")
            nc.vector.scalar_tensor_tensor(
                out=sl0[:], in0=sgn[:], scalar=1.0, in1=t1[:],
                op0=ALU.mult, op1=ALU.mult,
            )
            sig = small.tile([B_LOC, P], F32, tag="sig")
            nc.gpsimd.tensor_add(sig[:], sl0[:], t2[:])
            u = small.tile([B_LOC, P], F32, tag="u")
            nc.gpsimd.tensor_sub(u[:], tt_, sig[:])
            au = small.tile([B_LOC, P], F32, tag="au")
            nc.vector.scalar_tensor_tensor(
                out=au[:], in0=u[:], scalar=-1.0, in1=u[:],
                op0=ALU.mult, op1=ALU.max,
            )
            pt = small.tile([B_LOC, P], F32, tag="pt")
            nc.vector.tensor_scalar(pt[:], au[:], -1.0, 1.0, op0=ALU.mult, op1=ALU.add)
            au2 = small.tile([B_LOC, P], F32, tag="au2")
            nc.gpsimd.tensor_mul(au2[:], au[:], au[:])

            lnz = small.tile([B_LOC, 1], F32, tag="lnz")
            nc.scalar.activation(lnz[:], se[:], ACTF.Ln)
            nc.vector.tensor_copy(outsm[:, 1:2], lnz[:])
            lnpt = small.tile([B_LOC, P], F32, tag="lnpt")
            nc.scalar.activation(lnpt[:], pt[:], ACTF.Ln)
            junk20 = small.tile([B_LOC, P], F32, tag="junk20")
            fr = small.tile([B_LOC, 1], F32, tag="fr")
            # accum = sum(au^2 * ln(p_t)) = -focal_sum   (host negates)
            nc.vector.scalar_tensor_tensor(
                out=junk20[:], in0=au2[:], scalar=1.0, in1=lnpt[:],
                op0=ALU.mult, op1=ALU.mult, accum_out=fr[:],
            )
            nc.vector.tensor_copy(outsm[:, 2:3], fr[:])
            nc.sync.dma_start(out_sm[:, :], outsm[:])

            # ---- remaining heavy pieces ----
            for i in range(3, len(PIECES)):
                heavy(i)

            # ---- PE transpose of sums -> PSUM [NSUM, 128], tiny out DMA ----
            sums_t = accp.tile([NSUM, 128], F32, tag="sums_t")
            psum_t2 = ps.tile([2, 128], F32, tag="pt2")
            nc.tensor.matmul(psum_t[0:12, :], sums[:, 0:12], ident[:, :])
            nc.vector.tensor_copy(sums_t[0:12, :], psum_t[0:12, :])
            nc.sync.dma_start(out_hm[0:12, :], sums_t[0:12, :])
            sums_t2 = accp.tile([2, 128], F32, tag="sums_t2")
            nc.tensor.matmul(psum_t2[0:2, :], sums[:, 12:NSUM], ident[:, :])
            nc.vector.tensor_copy(sums_t2[0:2, :], psum_t2[0:2, :])
            nc.sync.dma_start(out_hm[12:NSUM, :], sums_t2[0:2, :])

    nc.compile()
    return nc


_MODULE = None


def _module():
    global _MODULE
    if _MODULE is None:
        _MODULE = build_module()
    return _MODULE


def make_in_maps(count_logits, pred_heatmaps, pred_conf_logits, gt_heatmaps,
                 count, mask):
    count_logits = np.asarray(count_logits, np.float32)
    pred_heatmaps = np.asarray(pred_heatmaps, np.float32)
    pred_conf_logits = np.asarray(pred_conf_logits, np.float32)
    gt_heatmaps = np.asarray(gt_heatmaps, np.float32)
    count = np.asarray(count, np.int32)
    mask = np.asarray(mask, np.int32)

    in_maps = []
    for i in range(N_CORES):
        b0, b1 = i * B_LOC, (i + 1) * B_LOC
        phl = pred_heatmaps[b0:b1].reshape(ROWS, COLS).astype(np.float16)
        ghl = gt_heatmaps[b0:b1].reshape(ROWS, COLS).astype(np.float16)

        im = {}
        for name, rr, cc, _ in PIECES:
            if name == "t":
                # rows 640:680 folded: partition h*40+r <-> row 640+r, half h
                pt_ = phl[640:].reshape(REM, 2, COLS // 2).transpose(1, 0, 2)
                gt_ = ghl[640:].reshape(REM, 2, COLS // 2).transpose(1, 0, 2)
                im["pt"] = np.ascontiguousarray(pt_.reshape(2 * REM, COLS // 2))
                im["gt"] = np.ascontiguousarray(gt_.reshape(2 * REM, COLS // 2))
            else:
                r0 = 512 if name in ("c4a", "c4b") else int(name[1]) * 128
                c0 = 0
                if name == "c4b":
                    c0 = 3584
                im["p" + name] = np.ascontiguousarray(phl[r0 : r0 + rr, c0 : c0 + cc])
                im["g" + name] = np.ascontiguousarray(ghl[r0 : r0 + rr, c0 : c0 + cc])

        smv = np.zeros((B_LOC, 84), np.float32)
        smv[np.arange(B_LOC), 21 + count[b0:b1]] = 1.0       # one-hot
        smv[:, 0:21] = count_logits[b0:b1]
        smv[:, 42:62] = pred_conf_logits[b0:b1]
        smv[:, 62:82] = mask[b0:b1].astype(np.float32)
        im["sm"] = smv
        in_maps.append(im)
    return in_maps


def _rowsums(hm):
    """[NSUM, 128] transposed sums -> [680] per-row weighted sums.

    Piece i's Sum(d^2) is row 2i, Sum(4*step*d^2) is row 2i+1; their sum
    is the per-(piece, partition) weighted heatmap contribution."""
    s = hm[0::2] + hm[1::2]          # [7, 128]
    rows = np.zeros(ROWS)
    rows[640:680] = s[0, :80].reshape(2, REM).sum(axis=0)
    for k in range(4):
        rows[128 * k : 128 * (k + 1)] = s[1 + k]
    rows[512:640] = s[5] + s[6]
    return rows


def combine(results, mask):
    mask = np.asarray(mask)
    hm_sum = 0.0
    ce_sum = 0.0
    fo_sum = 0.0
    for i, res in enumerate(results):
        b0, b1 = i * B_LOC, (i + 1) * B_LOC
        rowsum = _rowsums(np.asarray(res["out_hm"], np.float64))
        mrow = np.repeat(mask[b0:b1].astype(np.float64).reshape(-1), K)
        hm_sum += float(rowsum @ mrow)
        sm = np.asarray(res["out_sm"], np.float64)       # [2,3]: pre, lnz, fr
        ce_sum += float(sm[:, 0:2].sum())
        fo_sum += -float(sm[:, 2].sum())
    msum = float(mask.sum())
    hm = hm_sum / (msum * K * H * W + EPS)
    loss_heatmap = hm if msum > 0 else 0.0
    loss_count = ce_sum / B
    loss_conf = fo_sum / (B * P)
    total = (ALPHA_COUNT * loss_count + ALPHA_HEATMAP * loss_heatmap
             + ALPHA_CONF * loss_conf)
    return np.float32(total)


def run(inputs, trace=False, **kwargs):
    """Run on hardware; returns (output_scalar, BassKernelResults)."""
    nc = _module()
    in_maps = make_in_maps(**inputs)
    res = bass_utils.run_bass_kernel_spmd(
        nc, in_maps, core_ids=list(range(N_CORES)), trace=trace, **kwargs
    )
    out = combine(res.results, inputs["mask"])
    return out, res


def kernel(count_logits, pred_heatmaps, pred_conf_logits, gt_heatmaps,
           count, mask):
    out, _ = run(dict(
        count_logits=count_logits, pred_heatmaps=pred_heatmaps,
        pred_conf_logits=pred_conf_logits, gt_heatmaps=gt_heatmaps,
        count=count, mask=mask,
    ))
    return out


# revision 23
# speedup vs baseline: 1.3879x; 1.0454x over previous
"""End2EndPoseLoss on 8 Trainium2 NeuronCores — v2.

Data-parallel over batch: each core handles B_LOC=2 samples, i.e. a
[680, 4096] fp16 pred/gt pair (11.14 MB -> ~28.6 us DMA floor at the
measured ~390 GB/s per-core aggregate).  v2 is designed to be DMA-bound:

Per row-piece (5 full [128,4096] chunks, a [128,3584]+[128,512] split
of the last full rows, and the 40 leftover rows folded to [80,2048]):

  DVE : W4 = (g > 0.2) * 4          tensor_scalar, 4x perf mode
  DVE : d  = p - g                  tensor_tensor, 2x perf mode
  ACT : D2 = Square(d)  (in-place, full output, no accumulator)
  DVE/Pool : S = rowsum((W4+1)*D2)  scalar_tensor_tensor + accum_out,
             column-split ~25/75 between DVE (1x) and the otherwise
             idle GpSimd/Pool engine (0.6 eff) -> two sums columns.

This uses exact {1,5} weights (vs sqrt(5) folding) and puts every
engine just under the DMA roofline (ACT ~26us, DVE ~28, Pool ~27).

Output path: the [128,13] f32 per-(piece,engine) row sums are
transposed on the idle PE (matmul against an on-chip identity) into
PSUM [13,128], so the final DMA is 13 512-B descriptors instead of
128 24-B ones (the baseline lost ~7 us draining those semaphores).

Small losses (count CE over [2,21], conf focal over [2,20]) arrive as
one packed [2,84] tensor, run on DVE/ACT/Pool inside the DMA ramp, and
leave as one [2,3] tensor.
"""

import sys
import types
import numpy as np

import concourse.bacc as bacc
import concourse.bass as bass  # noqa: F401
import concourse.mybir as mybir
import concourse.tile as tile
from concourse import bass_utils

# Problem constants (hardcoded per contract).
B, P, K, H, W = 16, 20, 17, 64, 64
N_CORES = 8
B_LOC = B // N_CORES            # 2
ROWS = B_LOC * P * K            # 680
COLS = H * W                    # 4096
REM = 40                        # 680 - 5*128

PEAK_THRESH = 0.2
PEAK_WEIGHT = 5.0
ALPHA_COUNT, ALPHA_HEATMAP, ALPHA_CONF = 1.0, 10.0, 1.5
EPS = 1e-6

F32 = mybir.dt.float32
F16 = mybir.dt.float16
ALU = mybir.AluOpType
ACTF = mybir.ActivationFunctionType
AX = mybir.AxisListType

# pieces: (name, rows, cols, s2_dve = Sum2 via DVE stt instead of ACT square)
# piece i accumulates Sum(d^2) in column 2i and 4*Sum(step*d^2) in 2i+1.
PIECES = [
    ("t",  80, 2048,  False),
    ("c0", 128, 4096, False),
    ("c1", 128, 4096, False),
    ("c2", 128, 4096, False),
    ("c3", 128, 4096, True),
    ("c4a", 128, 3584, True),
    ("c4b", 128, 512,  False),
]
NSUM = 14


def _install_ntff_hook():
    """Provide antenv.axon_hooks if the image lacks it, so that
    run_bass_kernel_spmd(trace=True) doesn't crash and, when possible,
    actually profiles via the axon .so."""
    try:
        from antenv.axon_hooks import get_axon_ntff_profile_hook  # noqa: F401
        return
    except ImportError:
        pass
    try:
        import antenv
    except ImportError:
        return
    import contextlib
    import ctypes

    mod = types.ModuleType("antenv.axon_hooks")
    _h = [None]
    mod.set_axon_ntff_profile_hook = lambda h: _h.__setitem__(0, h)
    mod.get_axon_ntff_profile_hook = lambda: _h[0]
    sys.modules["antenv.axon_hooks"] = mod
    antenv.axon_hooks = mod

    so_path = "/opt/axon/libaxon_pjrt.so"
    try:
        lib = ctypes.CDLL(so_path)
        if not hasattr(lib, "axon_start_nrt_profile"):
            return
        lib.axon_start_nrt_profile.argtypes = [
            ctypes.POINTER(ctypes.c_int64),
            ctypes.c_size_t,
        ]
        lib.axon_start_nrt_profile.restype = ctypes.c_int64
        lib.axon_stop_nrt_profile.argtypes = [ctypes.c_char_p]
        lib.axon_stop_nrt_profile.restype = ctypes.c_int64
    except OSError:
        return

    @contextlib.contextmanager
    def _hook(output_dir, device_ids):
        import jax

        jax.devices()
        if device_ids:
            ids = (ctypes.c_int64 * len(device_ids))(*device_ids)
            rc = lib.axon_start_nrt_profile(ids, len(device_ids))
        else:
            rc = lib.axon_start_nrt_profile(None, 0)
        if rc != 0:
            raise RuntimeError(f"axon_start_nrt_profile rc={rc}")
        try:
            yield
        finally:
            n = lib.axon_stop_nrt_profile(str(output_dir).encode())
            print(f"profile: {n} file(s) written to {output_dir}", file=sys.stderr)

    mod.set_axon_ntff_profile_hook(_hook)


_install_ntff_hook()

_orig_upload = bass_utils.upload_artifacts


def _safe_upload(tmpdir):
    try:
        return _orig_upload(tmpdir)
    except Exception:
        return tmpdir


bass_utils.upload_artifacts = _safe_upload


def build_module():
    nc = bacc.Bacc("TRN2", target_bir_lowering=False, debug=False)

    srcs = {}
    for name, rr, cc, _ in PIECES:
        srcs["g" + name] = nc.dram_tensor("g" + name, [rr, cc], F16, kind="ExternalInput")
        srcs["p" + name] = nc.dram_tensor("p" + name, [rr, cc], F16, kind="ExternalInput")
    sm = nc.dram_tensor("sm", [B_LOC, 84], F32, kind="ExternalInput")

    out_hm = nc.dram_tensor("out_hm", [NSUM, 128], F32, kind="ExternalOutput")
    out_sm = nc.dram_tensor("out_sm", [B_LOC, 3], F32, kind="ExternalOutput")

    with tile.TileContext(nc) as tc:
        with (
            tc.tile_pool(name="io", bufs=5) as io,
            tc.tile_pool(name="wk", bufs=4) as wk,
            tc.tile_pool(name="acc", bufs=1) as accp,
            tc.tile_pool(name="small", bufs=1) as small,
            tc.tile_pool(name="ps", bufs=1, space=bass.MemorySpace.PSUM) as ps,
        ):
            sums = accp.tile([128, NSUM], F32, tag="sums")
            isrc = accp.tile([128, 128], F32, tag="isrc")
            ident = accp.tile([128, 128], F32, tag="ident")
            psum_t = ps.tile([NSUM, 128], F32, tag="pt")
            nc.gpsimd.memset(sums[:], 0.0)
            nc.gpsimd.memset(isrc[:], 1.0)
            nc.gpsimd.affine_select(
                out=ident[:], in_=isrc[:], pattern=[[-1, 128]],
                compare_op=ALU.is_equal, fill=0.0, base=0, channel_multiplier=1,
            )

            # ---- input DMA triggers, g before p so W4 can start early ----
            gts, pts = [], []
            for i, (name, rr, cc, _) in enumerate(PIECES):
                g = io.tile([128, COLS], F16, tag="g")
                p = io.tile([128, COLS], F16, tag="p")
                nc.sync.dma_start(g[:rr, :cc], srcs["g" + name][:, :])
                nc.sync.dma_start(p[:rr, :cc], srcs["p" + name][:, :])
                gts.append(g)
                pts.append(p)
                if i == 0:
                    smt = small.tile([B_LOC, 84], F32, tag="sm")
                    nc.sync.dma_start(smt[:], sm[:, :])

            def heavy(i):
                name, rr, cc, s2_dve = PIECES[i]
                c1, c2 = 2 * i, 2 * i + 1
                g, p = gts[i], pts[i]
                st = wk.tile([128, COLS], F16, tag="st")
                d = wk.tile([128, COLS], F16, tag="d")
                n = wk.tile([128, COLS], F16, tag="n")
                # step = (g > thresh)          (TS, 4x)
                nc.vector.tensor_scalar(
                    st[:rr, :cc], g[:rr, :cc], float(PEAK_THRESH), None,
                    op0=ALU.is_gt,
                )
                # d = p - g                    (TT, 2x)
                nc.vector.tensor_sub(d[:rr, :cc], p[:rr, :cc], g[:rr, :cc])
                # n = step * d                 (TT, 2x)
                nc.vector.tensor_mul(n[:rr, :cc], st[:rr, :cc], d[:rr, :cc])
                # Sum(d^2) on ACT (in-place square, accumulator -> col 2i)
                nc.scalar.activation(
                    d[:rr, :cc], d[:rr, :cc], ACTF.Square,
                    accum_out=sums[:rr, c1 : c1 + 1],
                )
                # 4*Sum((step*d)^2) -> col 2i+1: ACT square of 2n, or DVE stt
                if s2_dve:
                    nc.vector.scalar_tensor_tensor(
                        out=g[:rr, :cc], in0=n[:rr, :cc], scalar=4.0,
                        in1=n[:rr, :cc], op0=ALU.mult, op1=ALU.mult,
                        accum_out=sums[:rr, c2 : c2 + 1],
                    )
                else:
                    nc.scalar.activation(
                        n[:rr, :cc], n[:rr, :cc], ACTF.Square, scale=2.0,
                        accum_out=sums[:rr, c2 : c2 + 1],
                    )

            # first three pieces keep DVE/ACT fed through the ramp; the
            # small losses interleave after them
            heavy(0)
            heavy(1)
            heavy(2)

            # ---- small losses (inside the DMA stream) ----
            cl_t = smt[:, 0:21]
            oh_t = smt[:, 21:42]
            lt_ = smt[:, 42:62]
            tt_ = smt[:, 62:82]

            # count cross-entropy pieces
            mx = small.tile([B_LOC, 1], F32, tag="mx")
            nc.vector.tensor_reduce(mx[:], cl_t, axis=AX.X, op=ALU.max)
            nmx = small.tile([B_LOC, 1], F32, tag="nmx")
            nc.vector.tensor_scalar_mul(nmx[:], mx[:], -1.0)
            et = small.tile([B_LOC, 21], F32, tag="et")
            se = small.tile([B_LOC, 1], F32, tag="se")
            nc.scalar.activation(
                et[:], cl_t, ACTF.Exp, bias=nmx[:], scale=1.0, accum_out=se[:]
            )
            junk21 = small.tile([B_LOC, 21], F32, tag="junk21")
            tg = small.tile([B_LOC, 1], F32, tag="tg")
            nc.vector.scalar_tensor_tensor(
                out=junk21[:], in0=cl_t, scalar=1.0, in1=oh_t,
                op0=ALU.mult, op1=ALU.mult, accum_out=tg[:],
            )
            outsm = small.tile([B_LOC, 3], F32, tag="outsm")
            nc.vector.tensor_sub(outsm[:, 0:1], mx[:], tg[:])

            # focal: p_t = 1 - |t - sigma(l)| with sigma from exp(-|l|)
            ab = small.tile([B_LOC, P], F32, tag="ab")
            nc.vector.scalar_tensor_tensor(
                out=ab[:], in0=lt_, scalar=-1.0, in1=lt_,
                op0=ALU.mult, op1=ALU.max,
            )
            z = small.tile([B_LOC, P], F32, tag="z")
            nc.scalar.activation(z[:], ab[:], ACTF.Exp, scale=-1.0)
            zz = small.tile([B_LOC, P], F32, tag="zz")
            nc.vector.tensor_scalar(zz[:], z[:], 1.0, None, op0=ALU.add)
            r = small.tile([B_LOC, P], F32, tag="r")
            nc.vector.reciprocal(r[:], zz[:])          # sigma(|l|)
            sgn = small.tile([B_LOC, P], F32, tag="sgn")
            nc.vector.tensor_scalar(sgn[:], lt_, 0.0, None, op0=ALU.is_ge)
            t1 = small.tile([B_LOC, P], F32, tag="t1")
            nc.vector.tensor_scalar(t1[:], r[:], 2.0, -1.0, op0=ALU.mult, op1=ALU.add)
            t2 = small.tile([B_LOC, P], F32, tag="t2")
            nc.vector.tensor_scalar(t2[:], r[:], -1.0, 1.0, op0=ALU.mult, op1=ALU.add)
            sl0 = small.tile([B_LOC, P], F32, tag="sl0")
            nc.vector.scalar_tensor_tensor(
                out=sl0[:], in0=sgn[:], scalar=1.0, in1=t1[:],
                op0=ALU.mult, op1=ALU.mult,
            )
            sig = small.tile([B_LOC, P], F32, tag="sig")
            nc.gpsimd.tensor_add(sig[:], sl0[:], t2[:])
            u = small.tile([B_LOC, P], F32, tag="u")
            nc.gpsimd.tensor_sub(u[:], tt_, sig[:])
            au = small.tile([B_LOC, P], F32, tag="au")
            nc.vector.scalar_tensor_tensor(
                out=au[:], in0=u[:], scalar=-1.0, in1=u[:],
                op0=ALU.mult, op1=ALU.max,
            )
            pt = small.tile([B_LOC, P], F32, tag="pt")
            nc.vector.tensor_scalar(pt[:], au[:], -1.0, 1.0, op0=ALU.mult, op1=ALU.add)
            au2 = small.tile([B_LOC, P], F32, tag="au2")
            nc.gpsimd.tensor_mul(au2[:], au[:], au[:])

            lnz = small.tile([B_LOC, 1], F32, tag="lnz")
            nc.scalar.activation(lnz[:], se[:], ACTF.Ln)
            nc.vector.tensor_copy(outsm[:, 1:2], lnz[:])
            lnpt = small.tile([B_LOC, P], F32, tag="lnpt")
            nc.scalar.activation(lnpt[:], pt[:], ACTF.Ln)
            junk20 = small.tile([B_LOC, P], F32, tag="junk20")
            fr = small.tile([B_LOC, 1], F32, tag="fr")
            # accum = sum(au^2 * ln(p_t)) = -focal_sum   (host negates)
            nc.vector.scalar_tensor_tensor(
                out=junk20[:], in0=au2[:], scalar=1.0, in1=lnpt[:],
                op0=ALU.mult, op1=ALU.mult, accum_out=fr[:],
            )
            nc.vector.tensor_copy(outsm[:, 2:3], fr[:])
            nc.sync.dma_start(out_sm[:, :], outsm[:])

            # ---- remaining heavy pieces ----
            for i in range(3, len(PIECES)):
                heavy(i)

            # ---- PE transpose of sums -> PSUM [NSUM, 128], tiny out DMA ----
            sums_t = accp.tile([NSUM, 128], F32, tag="sums_t")
            psum_t2 = ps.tile([2, 128], F32, tag="pt2")
            nc.tensor.matmul(psum_t[0:12, :], sums[:, 0:12], ident[:, :])
            nc.vector.tensor_copy(sums_t[0:12, :], psum_t[0:12, :])
            nc.sync.dma_start(out_hm[0:12, :], sums_t[0:12, :])
            sums_t2 = accp.tile([2, 128], F32, tag="sums_t2")
            nc.tensor.matmul(psum_t2[0:2, :], sums[:, 12:NSUM], ident[:, :])
            nc.vector.tensor_copy(sums_t2[0:2, :], psum_t2[0:2, :])
            nc.sync.dma_start(out_hm[12:NSUM, :], sums_t2[0:2, :])

    nc.compile()
    return nc


_MODULE = None


def _module():
    global _MODULE
    if _MODULE is None:
        _MODULE = build_module()
    return _MODULE


def make_in_maps(count_logits, pred_heatmaps, pred_conf_logits, gt_heatmaps,
                 count, mask):
    count_logits = np.asarray(count_logits, np.float32)
    pred_heatmaps = np.asarray(pred_heatmaps, np.float32)
    pred_conf_logits = np.asarray(pred_conf_logits, np.float32)
    gt_heatmaps = np.asarray(gt_heatmaps, np.float32)
    count = np.asarray(count, np.int32)
    mask = np.asarray(mask, np.int32)

    in_maps = []
    for i in range(N_CORES):
        b0, b1 = i * B_LOC, (i + 1) * B_LOC
        phl = pred_heatmaps[b0:b1].reshape(ROWS, COLS).astype(np.float16)
        ghl = gt_heatmaps[b0:b1].reshape(ROWS, COLS).astype(np.float16)

        im = {}
        for name, rr, cc, _ in PIECES:
            if name == "t":
                # rows 640:680 folded: partition h*40+r <-> row 640+r, half h
                pt_ = phl[640:].reshape(REM, 2, COLS // 2).transpose(1, 0, 2)
                gt_ = ghl[640:].reshape(REM, 2, COLS // 2).transpose(1, 0, 2)
                im["pt"] = np.ascontiguousarray(pt_.reshape(2 * REM, COLS // 2))
                im["gt"] = np.ascontiguousarray(gt_.reshape(2 * REM, COLS // 2))
            else:
                r0 = 512 if name in ("c4a", "c4b") else int(name[1]) * 128
                c0 = 0
                if name == "c4b":
                    c0 = 3584
                im["p" + name] = np.ascontiguousarray(phl[r0 : r0 + rr, c0 : c0 + cc])
                im["g" + name] = np.ascontiguousarray(ghl[r0 : r0 + rr, c0 : c0 + cc])

        smv = np.zeros((B_LOC, 84), np.float32)
        smv[np.arange(B_LOC), 21 + count[b0:b1]] = 1.0       # one-hot
        smv[:, 0:21] = count_logits[b0:b1]
        smv[:, 42:62] = pred_conf_logits[b0:b1]
        smv[:, 62:82] = mask[b0:b1].astype(np.float32)
        im["sm"] = smv
        in_maps.append(im)
    return in_maps


def _rowsums(hm):
    """[NSUM, 128] transposed sums -> [680] per-row weighted sums.

    Piece i's Sum(d^2) is row 2i, Sum(4*step*d^2) is row 2i+1; their sum
    is the per-(piece, partition) weighted heatmap contribution."""
    s = hm[0::2] + hm[1::2]          # [7, 128]
    rows = np.zeros(ROWS)
    rows[640:680] = s[0, :80].reshape(2, REM).sum(axis=0)
    for k in range(4):
        rows[128 * k : 128 * (k + 1)] = s[1 + k]
    rows[512:640] = s[5] + s[6]
    return rows


def combine(results, mask):
    mask = np.asarray(mask)
    hm_sum = 0.0
    ce_sum = 0.0
    fo_sum = 0.0
    for i, res in enumerate(results):
        b0, b1 = i * B_LOC, (i + 1) * B_LOC
        rowsum = _rowsums(np.asarray(res["out_hm"], np.float64))
        mrow = np.repeat(mask[b0:b1].astype(np.float64).reshape(-1), K)
        hm_sum += float(rowsum @ mrow)
        sm = np.asarray(res["out_sm"], np.float64)       # [2,3]: pre, lnz, fr
        ce_sum += float(sm[:, 0:2].sum())
        fo_sum += -float(sm[:, 2].sum())
    msum = float(mask.sum())
    hm = hm_sum / (msum * K * H * W + EPS)
    loss_heatmap = hm if msum > 0 else 0.0
    loss_count = ce_sum / B
    loss_conf = fo_sum / (B * P)
    total = (ALPHA_COUNT * loss_count + ALPHA_HEATMAP * loss_heatmap
             + ALPHA_CONF * loss_conf)
    return np.float32(total)


def run(inputs, trace=False, **kwargs):
    """Run on hardware; returns (output_scalar, BassKernelResults)."""
    nc = _module()
    in_maps = make_in_maps(**inputs)
    res = bass_utils.run_bass_kernel_spmd(
        nc, in_maps, core_ids=list(range(N_CORES)), trace=trace, **kwargs
    )
    out = combine(res.results, inputs["mask"])
    return out, res


def kernel(count_logits, pred_heatmaps, pred_conf_logits, gt_heatmaps,
           count, mask):
    out, _ = run(dict(
        count_logits=count_logits, pred_heatmaps=pred_heatmaps,
        pred_conf_logits=pred_conf_logits, gt_heatmaps=gt_heatmaps,
        count=count, mask=mask,
    ))
    return out
